# revision 19
# baseline (speedup 1.0000x reference)
"""Causal single-head attention on 8 TRN2 NeuronCores.

Problem: x [4, 4096, 1024] f32, Wq/Wk/Wv [1024, 128] f32 ->
out [4, 4096, 128] f32 (causal softmax(QK^T/sqrt(128)) V).

Sharding: 2 cores per batch element; within a batch element the two
cores take interleaved query rows (even / odd). Each q tile of 128
interleaved rows spans 256 original rows, so tile j needs the kv prefix
0..256(j+1) — identical structure on every core (SPMD), causal waste
only ~6%.

SPMD parity trick: odd-parity cores receive xT with kv columns
pair-swapped (host-side permutation), so "my query rows" are always the
EVEN columns of xT — the Q projection reads xT with a stride-2 access
pattern that is identical on every core, and the causal masks (host
inputs) absorb the permutation. Attention is a sum over kv, so the kv
ordering is irrelevant elsewhere.

Streaming structure (per core, all matmuls bf16, fp32 PSUM):
  loop over 4 column-pairs (1024 seq cols each):
    DMA xT slices for this pair (split across both HWDGE rings),
    K.T blocks  = sum_c Wk_c.T @ xT_c
    V tiles     = sum_c xT_c,t.T @ Wv_c  (ones column appended)
    Q.T block   = sum_c Wq'_c.T @ xT_c[even cols]   (Wq' scaled 1/sqrt h)
    attention tiles of the PREVIOUS pair (so PE keeps working while this
    pair's xT streams in), two tiles at a time with interleaved chains:
      S.T chunks = (K.T chunk).T @ Q.T tile -> PSUM,
      P.T = exp(S.T) (ScalarE, no max subtraction - scores are O(5)),
      0/1 mask multiply on the two diagonal chunks,
      AV: out += (P.T chunk).T @ [V | 1], software-pipelined two stages
      behind the scores so ScalarE exp latency hides under PE work;
      last column is the softmax denominator (DVE reciprocal + scale).
"""
import sys

if "/opt/trn_rl_repo" not in sys.path:
    sys.path.insert(0, "/opt/trn_rl_repo")

import numpy as np
import ml_dtypes

SEQ, D, H = 4096, 1024, 128
# xT input segments (512-col granularity early for pipeline start, the
# final pair as one slab): (lo, hi) column ranges
XSEGS = ((0, 512), (512, 1024), (1024, 1536), (1536, 2048),
         (2048, 2560), (2560, 3072), (3072, 4096))
NQ = SEQ // 2        # q rows per core
NT = NQ // 128       # q tiles per core (16)
DC = D // 128        # d_model chunks (8)
SB = 512             # seq block for K psum regions
PAIR = 1024          # streaming granularity (seq cols)
N_CORES = 8

_BF16 = ml_dtypes.bfloat16
_CACHED_NC = None


def _build_nc():
    import concourse.bass as bass
    import concourse.bacc as bacc
    import concourse.mybir as mybir
    import concourse.tile as tile

    bf = mybir.dt.bfloat16
    f32 = mybir.dt.float32
    Exp = mybir.ActivationFunctionType.Exp

    nc = bacc.Bacc("TRN2", target_bir_lowering=False, debug=False,
                   num_devices=N_CORES)

    # xT arrives pre-packed partition-major per (segment, chunk-parity):
    # each input tensor is one fully contiguous DRAM read whose flat
    # element order matches the SBUF destination AP (p, c, w), so a whole
    # segment is a single maximally-bursting DMA per ring.
    xsegs_d = []
    for i, (lo, hi) in enumerate(XSEGS):
        pair = []
        for par in (0, 1):
            pair.append(nc.dram_tensor(f"x{i}{'eo'[par]}",
                                       [128, DC // 2, hi - lo], bf,
                                       kind="ExternalInput"))
        xsegs_d.append(pair)
    wq_d = nc.dram_tensor("wq", [128, DC * H], bf, kind="ExternalInput")
    wk_d = nc.dram_tensor("wk", [128, DC * H], bf, kind="ExternalInput")
    wv_d = nc.dram_tensor("wv", [128, DC * H], bf, kind="ExternalInput")
    m_d = nc.dram_tensor("masks", [128, 256], bf, kind="ExternalInput")
    out_d = nc.dram_tensor("out", [NQ, H], f32, kind="ExternalOutput")

    with tile.TileContext(nc) as tc:
        with (
            tc.tile_pool(name="const", bufs=1) as constp,
            tc.tile_pool(name="xbuf", bufs=1) as xbuf,
            tc.tile_pool(name="acts", bufs=1) as acts,
            tc.tile_pool(name="pt", bufs=5) as ptp,
            tc.tile_pool(name="ob", bufs=2) as obp,
            tc.tile_pool(name="rc", bufs=4) as rcp,
            tc.tile_pool(name="psum_kqv", bufs=2, space="PSUM") as psum_kqv,
            tc.tile_pool(name="psum_s", bufs=2, space="PSUM") as psum_s,
            tc.tile_pool(name="psum_o", bufs=2, space="PSUM") as psum_o,
        ):
            # ---- constants: wk first on scalar (needed by the first K
            # block); the rest on gpsimd so the sync/scalar rings carry
            # only the urgent pair-0 xT slices ----
            m_sb = constp.tile([128, 256], bf)
            wk_sb = constp.tile([128, DC, H], bf)
            wq_sb = constp.tile([128, DC, H], bf)
            wv_sb = constp.tile([128, DC, H], bf)
            nc.scalar.dma_start(wk_sb[:], wk_d[:, :])
            nc.gpsimd.dma_start(wv_sb[:], wv_d[:, :])
            nc.gpsimd.dma_start(wq_sb[:], wq_d[:, :])
            nc.gpsimd.dma_start(m_sb[:], m_d[:, :])
            m0 = m_sb[:, 0:128]
            m1 = m_sb[:, 128:256]

            # PE warmup: throwaway matmuls on a memset tile (no DMA
            # dependency) so the HAM clock-gate opens and PE stays warm
            # until the first xT data lands
            wtile = constp.tile([128, 640], bf)
            nc.vector.memset(wtile[:], 0.25)
            pw = psum_s.tile([128, 1024], f32, tag="ps")
            for _ in range(16):
                nc.tensor.matmul(pw[:, 0:512], wtile[:, 0:128],
                                 wtile[:, 128:640], start=True, stop=True)

            # preload the ACT exp table while DMAs stream
            warm = rcp.tile([128, 1], f32)
            nc.scalar.activation(warm[:], wtile[:, 0:1], Exp)

            xT = xbuf.tile([128, DC, SEQ], bf)
            KT = acts.tile([128, SEQ], bf)
            QT = acts.tile([128, NQ], bf)
            V = acts.tile([128, SEQ // 128, H + 1], bf)
            nc.vector.memset(V[:, :, H:H + 1], 1.0)

            # all xT DMAs up-front: both HWDGE rings crunch through them
            # back-to-back, decoupled from the compute emission order.
            # One DMA per (segment, chunk-parity): even chunks on sync,
            # odd on scalar; each source is fully contiguous.
            for i, (lo, hi) in enumerate(XSEGS):
                nc.sync.dma_start(xT[:, 0:DC:2, lo:hi], xsegs_d[i][0][:])
                nc.scalar.dma_start(xT[:, 1:DC:2, lo:hi], xsegs_d[i][1][:])

            def v_tile(t):
                pv = psum_kqv.tile([128, SB], f32, tag="kqv")
                for c in range(DC):
                    nc.tensor.matmul(pv[:, 0:H],
                                     xT[:, c, 128 * t:128 * (t + 1)],
                                     wv_sb[:, c, :],
                                     start=(c == 0), stop=(c == DC - 1))
                nc.vector.tensor_copy(V[:, t, 0:H], pv[:, 0:H])

            def av_group(stage):
                # AV matmuls for one stage: shared slots feed both tiles of
                # the pair (first/second 128 columns), extra slots only the
                # second tile. Chunk order ascends across stages so the
                # start/stop accumulation flags line up per tile.
                slots, ppt, po, po2, Cs = stage
                for kind, c, off in slots:
                    if kind == 's':
                        nc.tensor.matmul(po[:], ppt[:, off:off + 128],
                                         V[:, c, :],
                                         start=(c == 0), stop=(c == Cs - 1),
                                         skip_group_check=True)
                        nc.tensor.matmul(po2[:], ppt[:, off + 128:off + 256],
                                         V[:, c, :],
                                         start=(c == 0), stop=False,
                                         skip_group_check=True)
                    else:
                        nc.tensor.matmul(po2[:], ppt[:, off:off + 128],
                                         V[:, c, :],
                                         start=False, stop=(c == Cs + 1),
                                         skip_group_check=True)

            def emit_attention(Pa, fillers=None):
                # attention tiles for pair Pa, two tiles at a time. The two
                # tiles of a pair share every score stationary (KT chunk)
                # via a 256-wide moving operand, halving the LDWEIGHTS tax
                # on the shared prefix. AV is deferred two stages so ScalarE
                # exp latency hides under PE work. `fillers` are extra PE
                # work (V-tile projections) injected one per stage.
                for ja in range(4 * Pa, 4 * Pa + 4, 2):
                    j, j2 = ja, ja + 1
                    Cs = 2 * (j + 1)     # shared chunks = tile j's full set
                    po = psum_o.tile([128, H + 1], f32, tag="po")
                    po2 = psum_o.tile([128, H + 1], f32, tag="po")
                    stages = []
                    c = 0
                    while c < Cs - 2:
                        n = min(4, Cs - 2 - c)
                        stages.append([('s', cc) for cc in range(c, c + n)])
                        c += n
                    stages.append([('s', Cs - 2), ('s', Cs - 1),
                                   ('e', Cs), ('e', Cs + 1)])
                    pending = []
                    for stage in stages:
                        ps = psum_s.tile([128, 1024], f32, tag="ps")
                        slots = []
                        off = 0
                        for kind, cc in stage:
                            if kind == 's':
                                nc.tensor.matmul(ps[:, off:off + 256],
                                                 KT[:, 128 * cc:128 * (cc + 1)],
                                                 QT[:, 128 * j:128 * j + 256],
                                                 start=True, stop=True)
                                slots.append(('s', cc, off))
                                off += 256
                            else:
                                nc.tensor.matmul(ps[:, off:off + 128],
                                                 KT[:, 128 * cc:128 * (cc + 1)],
                                                 QT[:, 128 * j2:128 * (j2 + 1)],
                                                 start=True, stop=True)
                                slots.append(('e', cc, off))
                                off += 128
                        pt = ptp.tile([128, 1024], bf, tag="pt")
                        nc.scalar.activation(pt[:, 0:off], ps[:, 0:off], Exp)
                        for kind, cc, soff in slots:
                            if kind == 's' and cc == Cs - 2:
                                nc.vector.tensor_mul(
                                    pt[:, soff:soff + 128],
                                    pt[:, soff:soff + 128], m0)
                            elif kind == 's' and cc == Cs - 1:
                                nc.vector.tensor_mul(
                                    pt[:, soff:soff + 128],
                                    pt[:, soff:soff + 128], m1)
                            elif kind == 'e':
                                m = m0 if cc == Cs else m1
                                nc.vector.tensor_mul(
                                    pt[:, soff:soff + 128],
                                    pt[:, soff:soff + 128], m)
                        if fillers:
                            fillers.pop(0)()
                        pending.append((slots, pt, po, po2, Cs))
                        if len(pending) > 3:
                            av_group(pending.pop(0))
                    for st in pending:
                        av_group(st)
                    for jj, poo in ((j, po), (j2, po2)):
                        rc = rcp.tile([128, 1], f32, tag="rc")
                        nc.vector.reciprocal(rc[:], poo[:, H:H + 1])
                        ob = obp.tile([128, H], f32, tag="ob")
                        nc.vector.tensor_scalar_mul(ob[:], poo[:, 0:H], rc[:])
                        oeng = nc.sync if jj % 2 == 0 else nc.gpsimd
                        oeng.dma_start(out_d[128 * jj:128 * (jj + 1), :], ob[:])

            for P in range(SEQ // PAIR):
                lo = PAIR * P
                # attention of the previous pair first: its inputs are
                # ready, so PE stays busy while this pair's xT streams in
                if P > 0:
                    emit_attention(P - 1)
                # K.T blocks
                for s in (2 * P, 2 * P + 1):
                    b0 = SB * s
                    pk = psum_kqv.tile([128, SB], f32, tag="kqv")
                    for c in range(DC):
                        nc.tensor.matmul(pk[:], wk_sb[:, c, :],
                                         xT[:, c, b0:b0 + SB],
                                         start=(c == 0), stop=(c == DC - 1))
                    nc.vector.tensor_copy(KT[:, b0:b0 + SB], pk[:])
                # V tiles (last pair's become attention gap-fillers below)
                if P < SEQ // PAIR - 1:
                    for t in range(8 * P, 8 * P + 8):
                        v_tile(t)
                # Q.T block for this pair: even xT columns (stride 2)
                pq = psum_kqv.tile([128, SB], f32, tag="kqv")
                for c in range(DC):
                    nc.tensor.matmul(pq[:], wq_sb[:, c, :],
                                     xT[:, c, lo:lo + PAIR:2],
                                     start=(c == 0), stop=(c == DC - 1))
                nc.vector.tensor_copy(QT[:, SB * P:SB * (P + 1)], pq[:])

            lastP = SEQ // PAIR - 1
            emit_attention(lastP, fillers=[
                (lambda t=t: v_tile(t)) for t in range(8 * lastP, 8 * lastP + 8)])

    nc.compile()
    return nc


def _get_nc():
    global _CACHED_NC
    if _CACHED_NC is None:
        _CACHED_NC = _build_nc()
    return _CACHED_NC


def _pack_w(w):
    # [1024, 128] -> [128, 8*128] so SBUF tile [128, c, h] is one linear DMA
    return np.ascontiguousarray(
        w.reshape(DC, 128, H).transpose(1, 0, 2).reshape(128, DC * H))


def _make_in_maps(x, Wq, Wk, Wv):
    scale = np.float32(1.0 / np.sqrt(H))
    wq = _pack_w((np.asarray(Wq, np.float32) * scale).astype(_BF16))
    wk = _pack_w(np.asarray(Wk, np.float32).astype(_BF16))
    wv = _pack_w(np.asarray(Wv, np.float32).astype(_BF16))
    cc, ii = np.meshgrid(np.arange(128), np.arange(128), indexing="ij")
    masks = []
    for p in (0, 1):
        sig = cc + 1 - 2 * (cc % 2) if p == 1 else cc
        m0 = (sig <= 2 * ii + p).astype(np.float32)
        m1 = (sig + 128 <= 2 * ii + p).astype(np.float32)
        masks.append(np.concatenate([m0, m1], axis=1).astype(_BF16))
    in_maps = []
    for core in range(N_CORES):
        b, p = core // 2, core % 2
        xb = np.asarray(x[b], np.float32)
        if p == 1:
            # pair-swap kv rows so this core's q rows are the even ones
            xb = xb.reshape(SEQ // 2, 2, D)[:, ::-1, :].reshape(SEQ, D)
        xT = np.ascontiguousarray(xb.T).astype(_BF16)
        im = {"wq": wq, "wk": wk, "wv": wv, "masks": masks[p]}
        for i, (lo, hi) in enumerate(XSEGS):
            chunks = xT[:, lo:hi].reshape(DC, 128, hi - lo)
            for par in (0, 1):
                im[f"x{i}{'eo'[par]}"] = np.ascontiguousarray(
                    chunks[par::2].transpose(1, 0, 2))
        in_maps.append(im)
    return in_maps


def _run(x, Wq, Wk, Wv, trace=False):
    import time
    from concourse import bass_utils
    nc = _get_nc()
    in_maps = _make_in_maps(x, Wq, Wk, Wv)
    try:
        res = bass_utils.run_bass_kernel_spmd(
            nc, in_maps, core_ids=list(range(N_CORES)), trace=trace)
    except Exception:
        # transient NRT/device hiccups have been observed; retry once
        time.sleep(20)
        res = bass_utils.run_bass_kernel_spmd(
            nc, in_maps, core_ids=list(range(N_CORES)), trace=trace)
    out = np.empty((4, SEQ, H), np.float32)
    for core in range(N_CORES):
        out[core // 2, core % 2::2] = res.results[core]["out"]
    return out, res


def kernel(x, Wq, Wk, Wv):
    out, _ = _run(x, Wq, Wk, Wv, trace=False)
    return out



# revision 22
# speedup vs baseline: 1.0165x; 1.0165x over previous
"""Causal single-head attention on 8 TRN2 NeuronCores.

Problem: x [4, 4096, 1024] f32, Wq/Wk/Wv [1024, 128] f32 ->
out [4, 4096, 128] f32 (causal softmax(QK^T/sqrt(128)) V).

Sharding: 2 cores per batch element; within a batch element the two
cores take interleaved query rows (even / odd). Each q tile of 128
interleaved rows spans 256 original rows, so tile j needs the kv prefix
0..256(j+1) — identical structure on every core (SPMD), causal waste
only ~6%.

SPMD parity trick: odd-parity cores receive xT with kv columns
pair-swapped (host-side permutation), so "my query rows" are always the
EVEN columns of xT — the Q projection reads xT with a stride-2 access
pattern that is identical on every core, and the causal masks (host
inputs) absorb the permutation. Attention is a sum over kv, so the kv
ordering is irrelevant elsewhere.

Streaming structure (per core, all matmuls bf16, fp32 PSUM):
  loop over 4 column-pairs (1024 seq cols each):
    DMA xT slices for this pair (split across both HWDGE rings),
    K.T blocks  = sum_c Wk_c.T @ xT_c
    V tiles     = sum_c xT_c,t.T @ Wv_c  (ones column appended)
    Q.T block   = sum_c Wq'_c.T @ xT_c[even cols]   (Wq' scaled 1/sqrt h)
    attention tiles of the PREVIOUS pair (so PE keeps working while this
    pair's xT streams in), two tiles at a time with interleaved chains:
      S.T chunks = (K.T chunk).T @ Q.T tile -> PSUM,
      P.T = exp(S.T) (ScalarE, no max subtraction - scores are O(5)),
      0/1 mask multiply on the two diagonal chunks,
      AV: out += (P.T chunk).T @ [V | 1], software-pipelined two stages
      behind the scores so ScalarE exp latency hides under PE work;
      last column is the softmax denominator (DVE reciprocal + scale).
"""
import sys

if "/opt/trn_rl_repo" not in sys.path:
    sys.path.insert(0, "/opt/trn_rl_repo")

import numpy as np
import ml_dtypes

SEQ, D, H = 4096, 1024, 128
# xT input segments (512-col granularity early for pipeline start, the
# final pair as one slab): (lo, hi) column ranges
XSEGS = ((0, 512), (512, 1024), (1024, 1536), (1536, 2048),
         (2048, 2560), (2560, 3072), (3072, 4096))
NQ = SEQ // 2        # q rows per core
NT = NQ // 128       # q tiles per core (16)
DC = D // 128        # d_model chunks (8)
SB = 512             # seq block for K psum regions
PAIR = 1024          # streaming granularity (seq cols)
N_CORES = 8

_BF16 = ml_dtypes.bfloat16
_CACHED_NC = None


def _build_nc():
    import concourse.bass as bass
    import concourse.bacc as bacc
    import concourse.mybir as mybir
    import concourse.tile as tile

    bf = mybir.dt.bfloat16
    f32 = mybir.dt.float32
    Exp = mybir.ActivationFunctionType.Exp

    nc = bacc.Bacc("TRN2", target_bir_lowering=False, debug=False,
                   num_devices=N_CORES)

    # xT arrives pre-packed partition-major per (segment, chunk-parity):
    # each input tensor is one fully contiguous DRAM read whose flat
    # element order matches the SBUF destination AP (p, c, w), so a whole
    # segment is a single maximally-bursting DMA per ring.
    xsegs_d = [nc.dram_tensor(f"x{i}", [128, DC, hi - lo], bf,
                              kind="ExternalInput")
               for i, (lo, hi) in enumerate(XSEGS)]
    wq_d = nc.dram_tensor("wq", [128, DC * H], bf, kind="ExternalInput")
    wk_d = nc.dram_tensor("wk", [128, DC * H], bf, kind="ExternalInput")
    wv_d = nc.dram_tensor("wv", [128, DC * H], bf, kind="ExternalInput")
    m_d = nc.dram_tensor("masks", [128, 256], bf, kind="ExternalInput")
    out_d = nc.dram_tensor("out", [NQ, H], f32, kind="ExternalOutput")

    with tile.TileContext(nc) as tc:
        with (
            tc.tile_pool(name="const", bufs=1) as constp,
            tc.tile_pool(name="xbuf", bufs=1) as xbuf,
            tc.tile_pool(name="acts", bufs=1) as acts,
            tc.tile_pool(name="pt", bufs=5) as ptp,
            tc.tile_pool(name="ob", bufs=2) as obp,
            tc.tile_pool(name="rc", bufs=4) as rcp,
            tc.tile_pool(name="psum_kqv", bufs=2, space="PSUM") as psum_kqv,
            tc.tile_pool(name="psum_s", bufs=2, space="PSUM") as psum_s,
            tc.tile_pool(name="psum_o", bufs=2, space="PSUM") as psum_o,
        ):
            # ---- constants: wk first on scalar (needed by the first K
            # block); the rest on gpsimd so the sync/scalar rings carry
            # only the urgent pair-0 xT slices ----
            m_sb = constp.tile([128, 256], bf)
            wk_sb = constp.tile([128, DC, H], bf)
            wq_sb = constp.tile([128, DC, H], bf)
            wv_sb = constp.tile([128, DC, H], bf)
            nc.scalar.dma_start(wk_sb[:], wk_d[:, :])
            nc.gpsimd.dma_start(wv_sb[:], wv_d[:, :])
            nc.gpsimd.dma_start(wq_sb[:], wq_d[:, :])
            nc.gpsimd.dma_start(m_sb[:], m_d[:, :])
            m0 = m_sb[:, 0:128]
            m1 = m_sb[:, 128:256]

            # PE warmup: throwaway matmuls on a memset tile (no DMA
            # dependency) so the HAM clock-gate opens and PE stays warm
            # until the first xT data lands
            wtile = constp.tile([128, 640], bf)
            nc.vector.memset(wtile[:], 0.25)
            pw = psum_s.tile([128, 1024], f32, tag="ps")
            for _ in range(16):
                nc.tensor.matmul(pw[:, 0:512], wtile[:, 0:128],
                                 wtile[:, 128:640], start=True, stop=True)

            # preload the ACT exp table while DMAs stream
            warm = rcp.tile([128, 1], f32)
            nc.scalar.activation(warm[:], wtile[:, 0:1], Exp)

            xT = xbuf.tile([128, DC, SEQ], bf)
            KT = acts.tile([128, SEQ], bf)
            QT = acts.tile([128, NQ], bf)
            V = acts.tile([128, SEQ // 128, H + 1], bf)
            nc.vector.memset(V[:, :, H:H + 1], 1.0)

            # all xT DMAs up-front, one whole-segment DMA each, segments
            # round-robined over three DGE queues (the per-queue rate is
            # the bottleneck, ~130 GB/s); sources are fully contiguous.
            xrings = [nc.sync, nc.scalar, nc.gpsimd]
            for i, (lo, hi) in enumerate(XSEGS):
                xrings[i % 3].dma_start(xT[:, :, lo:hi], xsegs_d[i][:])

            def v_tile(t):
                pv = psum_kqv.tile([128, SB], f32, tag="kqv")
                for c in range(DC):
                    nc.tensor.matmul(pv[:, 0:H],
                                     xT[:, c, 128 * t:128 * (t + 1)],
                                     wv_sb[:, c, :],
                                     start=(c == 0), stop=(c == DC - 1))
                nc.vector.tensor_copy(V[:, t, 0:H], pv[:, 0:H])

            def av_group(stage):
                # AV matmuls for one stage: shared slots feed both tiles of
                # the pair (first/second 128 columns), extra slots only the
                # second tile. Chunk order ascends across stages so the
                # start/stop accumulation flags line up per tile.
                slots, ppt, po, po2, Cs = stage
                for kind, c, off in slots:
                    if kind == 's':
                        nc.tensor.matmul(po[:], ppt[:, off:off + 128],
                                         V[:, c, :],
                                         start=(c == 0), stop=(c == Cs - 1),
                                         skip_group_check=True)
                        nc.tensor.matmul(po2[:], ppt[:, off + 128:off + 256],
                                         V[:, c, :],
                                         start=(c == 0), stop=False,
                                         skip_group_check=True)
                    else:
                        nc.tensor.matmul(po2[:], ppt[:, off:off + 128],
                                         V[:, c, :],
                                         start=False, stop=(c == Cs + 1),
                                         skip_group_check=True)

            def emit_attention(Pa, fillers=None):
                # attention tiles for pair Pa, two tiles at a time. The two
                # tiles of a pair share every score stationary (KT chunk)
                # via a 256-wide moving operand, halving the LDWEIGHTS tax
                # on the shared prefix. AV is deferred two stages so ScalarE
                # exp latency hides under PE work. `fillers` are extra PE
                # work (V-tile projections) injected one per stage.
                for ja in range(4 * Pa, 4 * Pa + 4, 2):
                    j, j2 = ja, ja + 1
                    Cs = 2 * (j + 1)     # shared chunks = tile j's full set
                    po = psum_o.tile([128, H + 1], f32, tag="po")
                    po2 = psum_o.tile([128, H + 1], f32, tag="po")
                    stages = []
                    c = 0
                    while c < Cs - 2:
                        n = min(4, Cs - 2 - c)
                        stages.append([('s', cc) for cc in range(c, c + n)])
                        c += n
                    stages.append([('s', Cs - 2), ('s', Cs - 1),
                                   ('e', Cs), ('e', Cs + 1)])
                    pending = []
                    for stage in stages:
                        ps = psum_s.tile([128, 1024], f32, tag="ps")
                        slots = []
                        off = 0
                        for kind, cc in stage:
                            if kind == 's':
                                nc.tensor.matmul(ps[:, off:off + 256],
                                                 KT[:, 128 * cc:128 * (cc + 1)],
                                                 QT[:, 128 * j:128 * j + 256],
                                                 start=True, stop=True)
                                slots.append(('s', cc, off))
                                off += 256
                            else:
                                nc.tensor.matmul(ps[:, off:off + 128],
                                                 KT[:, 128 * cc:128 * (cc + 1)],
                                                 QT[:, 128 * j2:128 * (j2 + 1)],
                                                 start=True, stop=True)
                                slots.append(('e', cc, off))
                                off += 128
                        pt = ptp.tile([128, 1024], bf, tag="pt")
                        nc.scalar.activation(pt[:, 0:off], ps[:, 0:off], Exp)
                        for kind, cc, soff in slots:
                            if kind == 's' and cc == Cs - 2:
                                nc.vector.tensor_mul(
                                    pt[:, soff:soff + 128],
                                    pt[:, soff:soff + 128], m0)
                            elif kind == 's' and cc == Cs - 1:
                                nc.vector.tensor_mul(
                                    pt[:, soff:soff + 128],
                                    pt[:, soff:soff + 128], m1)
                            elif kind == 'e':
                                m = m0 if cc == Cs else m1
                                nc.vector.tensor_mul(
                                    pt[:, soff:soff + 128],
                                    pt[:, soff:soff + 128], m)
                        if fillers:
                            fillers.pop(0)()
                        pending.append((slots, pt, po, po2, Cs))
                        if len(pending) > 3:
                            av_group(pending.pop(0))
                    for st in pending:
                        av_group(st)
                    for jj, poo in ((j, po), (j2, po2)):
                        rc = rcp.tile([128, 1], f32, tag="rc")
                        nc.vector.reciprocal(rc[:], poo[:, H:H + 1])
                        ob = obp.tile([128, H], f32, tag="ob")
                        nc.vector.tensor_scalar_mul(ob[:], poo[:, 0:H], rc[:])
                        oeng = nc.sync if jj % 2 == 0 else nc.gpsimd
                        oeng.dma_start(out_d[128 * jj:128 * (jj + 1), :], ob[:])

            for P in range(SEQ // PAIR):
                lo = PAIR * P
                # attention of the previous pair first: its inputs are
                # ready, so PE stays busy while this pair's xT streams in
                if P > 0:
                    emit_attention(P - 1)
                # K.T blocks
                for s in (2 * P, 2 * P + 1):
                    b0 = SB * s
                    pk = psum_kqv.tile([128, SB], f32, tag="kqv")
                    for c in range(DC):
                        nc.tensor.matmul(pk[:], wk_sb[:, c, :],
                                         xT[:, c, b0:b0 + SB],
                                         start=(c == 0), stop=(c == DC - 1))
                    nc.vector.tensor_copy(KT[:, b0:b0 + SB], pk[:])
                # V tiles (last pair's become attention gap-fillers below)
                if P < SEQ // PAIR - 1:
                    for t in range(8 * P, 8 * P + 8):
                        v_tile(t)
                # Q.T block for this pair: even xT columns (stride 2)
                pq = psum_kqv.tile([128, SB], f32, tag="kqv")
                for c in range(DC):
                    nc.tensor.matmul(pq[:], wq_sb[:, c, :],
                                     xT[:, c, lo:lo + PAIR:2],
                                     start=(c == 0), stop=(c == DC - 1))
                nc.vector.tensor_copy(QT[:, SB * P:SB * (P + 1)], pq[:])

            lastP = SEQ // PAIR - 1
            emit_attention(lastP, fillers=[
                (lambda t=t: v_tile(t)) for t in range(8 * lastP, 8 * lastP + 8)])

    nc.compile()
    return nc


def _get_nc():
    global _CACHED_NC
    if _CACHED_NC is None:
        _CACHED_NC = _build_nc()
    return _CACHED_NC


def _pack_w(w):
    # [1024, 128] -> [128, 8*128] so SBUF tile [128, c, h] is one linear DMA
    return np.ascontiguousarray(
        w.reshape(DC, 128, H).transpose(1, 0, 2).reshape(128, DC * H))


def _make_in_maps(x, Wq, Wk, Wv):
    scale = np.float32(1.0 / np.sqrt(H))
    wq = _pack_w((np.asarray(Wq, np.float32) * scale).astype(_BF16))
    wk = _pack_w(np.asarray(Wk, np.float32).astype(_BF16))
    wv = _pack_w(np.asarray(Wv, np.float32).astype(_BF16))
    cc, ii = np.meshgrid(np.arange(128), np.arange(128), indexing="ij")
    masks = []
    for p in (0, 1):
        sig = cc + 1 - 2 * (cc % 2) if p == 1 else cc
        m0 = (sig <= 2 * ii + p).astype(np.float32)
        m1 = (sig + 128 <= 2 * ii + p).astype(np.float32)
        masks.append(np.concatenate([m0, m1], axis=1).astype(_BF16))
    in_maps = []
    for core in range(N_CORES):
        b, p = core // 2, core % 2
        xb = np.asarray(x[b], np.float32)
        if p == 1:
            # pair-swap kv rows so this core's q rows are the even ones
            xb = xb.reshape(SEQ // 2, 2, D)[:, ::-1, :].reshape(SEQ, D)
        xT = np.ascontiguousarray(xb.T).astype(_BF16)
        im = {"wq": wq, "wk": wk, "wv": wv, "masks": masks[p]}
        for i, (lo, hi) in enumerate(XSEGS):
            chunks = xT[:, lo:hi].reshape(DC, 128, hi - lo)
            im[f"x{i}"] = np.ascontiguousarray(chunks.transpose(1, 0, 2))
        in_maps.append(im)
    return in_maps


def _run(x, Wq, Wk, Wv, trace=False):
    import time
    from concourse import bass_utils
    nc = _get_nc()
    in_maps = _make_in_maps(x, Wq, Wk, Wv)
    try:
        res = bass_utils.run_bass_kernel_spmd(
            nc, in_maps, core_ids=list(range(N_CORES)), trace=trace)
    except Exception:
        # transient NRT/device hiccups have been observed; retry once
        time.sleep(20)
        res = bass_utils.run_bass_kernel_spmd(
            nc, in_maps, core_ids=list(range(N_CORES)), trace=trace)
    out = np.empty((4, SEQ, H), np.float32)
    for core in range(N_CORES):
        out[core // 2, core % 2::2] = res.results[core]["out"]
    return out, res


def kernel(x, Wq, Wk, Wv):
    out, _ = _run(x, Wq, Wk, Wv, trace=False)
    return out

